# revision 7
# baseline (speedup 1.0000x reference)
"""Trainium2 Bass kernel for nn_PartRefinement.

Strategy (pure data parallel, 4 samples per core x 8 cores):

The reference's bilinear sampling is degenerate: with integer-cast weights,
only w11 = (x2-x1)*(y2-y1) in {0,1} survives, so projection is a single-pixel
gather masked by {0,1}.  We store images transposed ([S*S+1, C], zero row at
index S*S) in HBM and use dma_gather with index = masked ? x1*S+y1 : S*S.
The gather output [128, 8, C] is exactly the lhsT layout (points on
partitions) needed by the fc matmul (fc contracts over the point axis).

Everything after the single relu folds into two matmuls on host:
  f1  = c1_w_perm @ [img_fc ; point ; grid ; fc_b] + (c1_cg @ codes + c1_b)
  f1r = relu(W32 @ f1 + b32)          W32 = c3_w @ c2_w
  feat= WF @ f1r + bF                 WF folds w2d1/w2d2/w2d3/c4/c5/c6 chain
"""
import sys
from contextlib import ExitStack

import numpy as np

try:
    from concourse import bass, bacc, mybir, tile
except ImportError:  # fresh env without the axon site paths
    sys.path.insert(0, "/opt/trn_rl_repo")
    from concourse import bass, bacc, mybir, tile

from concourse.bass_utils import run_bass_kernel_spmd

F32 = mybir.dt.float32
I32 = mybir.dt.int32
I16 = mybir.dt.int16

B, N = 32, 1024
NCORES = 8
BPC = B // NCORES                     # samples per core
IMG_SIZES = [56, 28, 14, 7]
IMG_CH = [64, 128, 256, 512]
NH = 512                              # moving free dim (half of N)

# fc m-chunk -> (gather scale, column offset, width) in kernel channel order
# [img1(128) | img2(256) | img3(512) | img0(64)+point(3)+grid(2)+fcb(1)]
MCHUNKS = [
    (1, 0, 128), (2, 0, 128), (2, 128, 128),
    (3, 0, 128), (3, 128, 128), (3, 256, 128), (3, 384, 128),
    (0, 0, 64),
]

LAST_RESULTS = None                   # BassKernelResults of the last run
TRACE = False


def build_nc():
    nc = bacc.Bacc("TRN2", target_bir_lowering=False, debug=False)

    lvl = nc.declare_dram_parameter("lvl", [BPC, 3, N], F32, isOutput=False)
    its = [
        nc.declare_dram_parameter(
            f"it{i}", [BPC, IMG_SIZES[i] * IMG_SIZES[i] + 1, IMG_CH[i]], F32,
            isOutput=False)
        for i in range(4)
    ]
    fcwT = nc.declare_dram_parameter("fcwT", [1024, 1024], F32, isOutput=False)
    c1wT = nc.declare_dram_parameter("c1wT", [966, 1024], F32, isOutput=False)
    cgT = nc.declare_dram_parameter("cgT", [2048, 1024], F32, isOutput=False)
    codesT = nc.declare_dram_parameter("codesT", [2048, BPC], F32, isOutput=False)
    c1b = nc.declare_dram_parameter("c1b", [1024], F32, isOutput=False)
    w32T = nc.declare_dram_parameter("w32T", [1024, 64], F32, isOutput=False)
    b32v = nc.declare_dram_parameter("b32v", [64], F32, isOutput=False)
    wfT = nc.declare_dram_parameter("wfT", [64, 6], F32, isOutput=False)
    bfv = nc.declare_dram_parameter("bfv", [6], F32, isOutput=False)
    grid2 = nc.declare_dram_parameter("grid2", [2, N], F32, isOutput=False)
    fcb = nc.declare_dram_parameter("fcb", [N], F32, isOutput=False)
    feat = nc.declare_dram_parameter("feat", [BPC, 6, N], F32, isOutput=True)

    add, mult, subtract = (mybir.AluOpType.add, mybir.AluOpType.mult,
                           mybir.AluOpType.subtract)
    is_gt = mybir.AluOpType.is_gt
    amax, amin = mybir.AluOpType.max, mybir.AluOpType.min

    with tile.TileContext(nc) as tc, ExitStack() as es:
        def pool(name, bufs, space="SBUF"):
            return es.enter_context(
                tc.tile_pool(name=name, bufs=bufs, space=space))

        wp = pool("weights", 1)
        cgp = pool("cgpool", 3)
        scr = pool("scratch", 1)
        gp = pool("gather", 2)
        yep = pool("yext", 1)
        f1sb = pool("f1sb", 1)
        outp = pool("outsb", 2)
        psA = pool("psA", 2, "PSUM")
        psB = pool("psB", 2, "PSUM")
        psC = pool("psC", 1, "PSUM")
        psD = pool("psD", 1, "PSUM")
        psE = pool("psE", 2, "PSUM")

        # ---- persistent weights -------------------------------------------
        fcw_t = wp.tile([128, 8, 1024], F32, name="fcw")
        nc.sync.dma_start(out=fcw_t[:], in_=fcwT[:].rearrange("(k p) o -> p k o", p=128))
        c1wa = wp.tile([128, 7, 1024], F32, name="c1wa")
        nc.sync.dma_start(out=c1wa[:],
                          in_=c1wT[0:896].rearrange("(k p) o -> p k o", p=128))
        c1wb = wp.tile([70, 1024], F32, name="c1wb")
        nc.sync.dma_start(out=c1wb[:], in_=c1wT[896:966])
        w32_t = wp.tile([128, 8, 64], F32, name="w32t")
        nc.sync.dma_start(out=w32_t[:], in_=w32T[:].rearrange("(k p) m -> p k m", p=128))
        wf_t = wp.tile([64, 6], F32, name="wft")
        nc.sync.dma_start(out=wf_t[:], in_=wfT[:])
        b32_t = wp.tile([64, 1], F32, name="b32t")
        nc.sync.dma_start(out=b32_t[:], in_=b32v[:].unsqueeze(1))
        bf_t = wp.tile([6, 1], F32, name="bft")
        nc.sync.dma_start(out=bf_t[:], in_=bfv[:].unsqueeze(1))
        c1b_t = wp.tile([128, 8], F32, name="c1bt")
        nc.sync.dma_start(out=c1b_t[:], in_=c1b[:].rearrange("(m p) -> p m", p=128))
        codes_t = wp.tile([128, 16, BPC], F32, name="codest")
        nc.sync.dma_start(out=codes_t[:],
                          in_=codesT[:].rearrange("(k p) b -> p k b", p=128))

        # ---- coordinates & gather indices ---------------------------------
        # c16[p, c, b, i] = level0[b, c, i*16 + p]  (dma_gather wrap order),
        # then replicated to all 8 groups of 16 partitions.
        c16 = scr.tile([16, 3, BPC, 64], F32, name="c16")
        for c in range(3):
            for b in range(BPC):
                nc.sync.dma_start(out=c16[:, c, b, :],
                                  in_=lvl[b, c].rearrange("(i p) -> p i", p=16))
        coords = scr.tile([128, 3, BPC, 64], F32, name="coords")
        for g in range(8):
            nc.sync.dma_start(out=coords[16 * g:16 * (g + 1)], in_=c16[:])

        NW = BPC * 64                                # free width of coord math
        x0 = coords[:, 0].rearrange("p b i -> p (b i)")
        yy = coords[:, 1].rearrange("p b i -> p (b i)")
        zz = coords[:, 2].rearrange("p b i -> p (b i)")

        def s_t(tag):
            return scr.tile([128, NW], F32, name=tag, tag=tag)

        rz = s_t("rz")
        nc.vector.reciprocal(rz[:], zz)
        hh = s_t("hh")
        ww = s_t("ww")
        tq = s_t("tq")
        tr = s_t("tr")
        for qt, num, dst, sgn in ((tq, yy, hh, 248.0), (tr, x0, ww, -248.0)):
            nc.vector.tensor_tensor(qt[:], num, rz[:], op=mult)     # q0 = n*rz
            # one Newton step of the division: q = q0 + rz*(n - q0*z)
            t1 = s_t("nt1")
            nc.vector.tensor_tensor(t1[:], qt[:], zz, op=mult)
            nc.vector.tensor_tensor(t1[:], num, t1[:], op=subtract)
            nc.vector.tensor_tensor(t1[:], t1[:], rz[:], op=mult)
            nc.vector.tensor_tensor(qt[:], qt[:], t1[:], op=add)
            nc.vector.tensor_scalar(out=dst[:], in0=qt[:], scalar1=sgn,
                                    scalar2=111.5, op0=mult, op1=add)
            nc.vector.tensor_scalar(out=dst[:], in0=dst[:], scalar1=0.0,
                                    scalar2=223.0, op0=amax, op1=amin)

        idx_tiles = []
        for s, S in enumerate(IMG_SIZES):
            fls = []
            ms = []
            for axis, src in ((0, hh), (1, ww)):
                xf = s_t("xf")
                nc.vector.tensor_scalar_mul(xf[:], src[:], float(S / 224.0))
                xi = scr.tile([128, NW], I32, name="xi", tag="xi")
                nc.vector.tensor_copy(xi[:], xf[:])                  # cast
                xq = s_t("xq")
                nc.vector.tensor_copy(xq[:], xi[:])                  # back to f32
                gt = s_t("gt")
                nc.vector.tensor_tensor(gt[:], xq[:], xf[:], op=is_gt)
                fl = s_t(f"fl{axis}")
                nc.vector.tensor_tensor(fl[:], xq[:], gt[:], op=subtract)
                hf = s_t("hf")
                nc.vector.tensor_tensor(hf[:], xf[:], fl[:], op=is_gt)
                x2 = s_t("x2")
                nc.vector.tensor_tensor(x2[:], fl[:], hf[:], op=add)
                nc.vector.tensor_scalar_min(x2[:], x2[:], float(S - 1))
                mx = s_t(f"mx{axis}")
                nc.vector.tensor_tensor(mx[:], x2[:], fl[:], op=subtract)
                fls.append(fl)
                ms.append(mx)
            idxf = s_t("idxf")
            nc.vector.tensor_scalar_mul(idxf[:], fls[0][:], float(S))
            nc.vector.tensor_tensor(idxf[:], idxf[:], fls[1][:], op=add)
            mm = s_t("mm")
            nc.vector.tensor_tensor(mm[:], ms[0][:], ms[1][:], op=mult)
            # idx_eff = m*(idx - S^2) + S^2
            nc.vector.tensor_scalar_sub(idxf[:], idxf[:], float(S * S))
            nc.vector.tensor_tensor(idxf[:], idxf[:], mm[:], op=mult)
            nc.vector.tensor_scalar_add(idxf[:], idxf[:], float(S * S))
            idx16 = scr.tile([128, BPC, 64], I16, name=f"idx{s}", tag=f"idx{s}")
            nc.vector.tensor_copy(idx16.rearrange("p b i -> p (b i)"), idxf[:])
            idx_tiles.append(idx16)

        # ---- gathers -------------------------------------------------------
        g_tiles = [None] * BPC

        def emit_gathers(b):
            tiles = []
            for s in range(4):
                C = IMG_CH[s]
                gt_ = gp.tile([128, 8, C], F32, name=f"g{s}_{b}", tag=f"g{s}")
                nc.gpsimd.dma_gather(
                    out_ap=gt_[:],
                    in_ap=its[s][b],
                    idxs_ap=idx_tiles[s][:, b, :],
                    num_idxs=N,
                    num_idxs_reg=N,
                    elem_size=C,
                    queue_num=0,
                )
                tiles.append(gt_)
            g_tiles[b] = tiles

        emit_gathers(0)
        emit_gathers(1)

        # ---- CT = c1_cg @ [code; global_code] + c1_b  (all samples) -------
        # PSUM holds only one k-step (per-matmul start/stop); accumulate in
        # SBUF to avoid 8 interleaved accumulation groups in one bank.
        ct_sb = wp.tile([128, 8, BPC], F32, name="ctsb")
        nc.vector.memset(ct_sb[:], 0.0)
        for k in range(16):
            cg_t = cgp.tile([128, 1024], F32, name=f"cg{k}", tag="cg")
            nc.sync.dma_start(out=cg_t[:], in_=cgT[128 * k:128 * (k + 1)])
            ctp = psE.tile([128, 8, BPC], F32, name="ctp", tag="ctp")
            for m in range(8):
                nc.tensor.matmul(
                    ctp[:, m, :], cg_t[:, 128 * m:128 * (m + 1)],
                    codes_t[:, k, :], start=True, stop=True,
                )
            acc_v = ct_sb.rearrange("p m b -> p (m b)")
            nc.vector.tensor_tensor(acc_v, acc_v,
                                    ctp.rearrange("p m b -> p (m b)"), op=add)
        for m in range(8):
            nc.vector.tensor_scalar(out=ct_sb[:, m, :], in0=ct_sb[:, m, :],
                                    scalar1=c1b_t[:, m:m + 1], scalar2=None,
                                    op0=add)

        # ---- per-sample pipeline ------------------------------------------
        for b in range(BPC):
            g0, g1, g2, g3 = g_tiles[b]
            gmap = {0: g0, 1: g1, 2: g2, 3: g3}
            for oc in range(2):
                osl = slice(oc * NH, (oc + 1) * NH)
                # fc: Y[c, o] = sum_n GT[n, c] fcwT[n, o]
                ye = [yep.tile([128, NH], F32, name=f"ye{k}", tag=f"ye{k}")
                      for k in range(7)]
                ye7 = yep.tile([70, NH], F32, name="ye7", tag="ye7")
                for m, (s, c0, cw) in enumerate(MCHUNKS):
                    yp = psA.tile([cw, NH], F32, name="yp", tag="yp")
                    for k in range(8):
                        nc.tensor.matmul(
                            yp[:], gmap[s][:, k, c0:c0 + cw], fcw_t[:, k, osl],
                            start=(k == 0), stop=(k == 7),
                        )
                    if m < 7:
                        nc.vector.tensor_copy(ye[m][:], yp[:])
                    else:
                        nc.vector.tensor_copy(ye7[0:64, :], yp[:])
                nc.sync.dma_start(out=ye7[64:67, :], in_=lvl[b][:, osl])
                nc.sync.dma_start(out=ye7[67:69, :], in_=grid2[:, osl])
                nc.sync.dma_start(out=ye7[69:70, :], in_=fcb[osl].unsqueeze(0))

                # c1
                f1 = [f1sb.tile([128, NH], F32, name=f"f1_{m}", tag=f"f1_{m}")
                      for m in range(8)]
                for m in range(8):
                    fp1 = psB.tile([128, NH], F32, name="f1p", tag="f1p")
                    for k in range(8):
                        if k < 7:
                            nc.tensor.matmul(
                                fp1[:], c1wa[:, k, 128 * m:128 * (m + 1)],
                                ye[k][:], start=(k == 0), stop=False)
                        else:
                            nc.tensor.matmul(
                                fp1[:], c1wb[:, 128 * m:128 * (m + 1)],
                                ye7[:], start=False, stop=True)
                    nc.vector.tensor_scalar(out=f1[m][:], in0=fp1[:],
                                            scalar1=ct_sb[:, m, b:b + 1],
                                            scalar2=None, op0=add)

                # W32 + relu
                rp = psC.tile([64, NH], F32, name="rp", tag="rp")
                for k in range(8):
                    nc.tensor.matmul(rp[:], w32_t[:, k, :], f1[k][:],
                                     start=(k == 0), stop=(k == 7))
                f1r = outp.tile([64, NH], F32, name="f1r", tag="f1r")
                nc.scalar.activation(f1r[:], rp[:],
                                     mybir.ActivationFunctionType.Relu,
                                     bias=b32_t[:], scale=1.0)

                # WF
                fpp = psD.tile([6, NH], F32, name="fpp", tag="fpp")
                nc.tensor.matmul(fpp[:], wf_t[:], f1r[:])
                feat_sb = outp.tile([6, NH], F32, name="featsb", tag="featsb")
                nc.vector.tensor_scalar(out=feat_sb[:], in0=fpp[:],
                                        scalar1=bf_t[:], scalar2=None, op0=add)
                nc.sync.dma_start(out=feat[b][:, osl], in_=feat_sb[:])

            if b + 2 < BPC:
                emit_gathers(b + 2)

    nc.compile()
    return nc


def fold_weights(inp):
    f64 = np.float64
    g = lambda k: np.asarray(inp[k], f64)
    w2d1s = g('w2d1').sum(-1)
    W21 = g('w2d2') @ w2d1s
    b21 = g('w2d2') @ g('b2d1') + g('b2d2')
    BD3 = np.zeros((64, 128), f64)
    for u in range(2):
        BD3[np.arange(32) * 2 + u, u * 64:(u + 1) * 64] = g('w2d3')
    b3x = np.zeros(64, f64)
    b3x[0::2] = g('b2d3'); b3x[1::2] = g('b2d3')
    W321 = BD3 @ W21
    b321 = BD3 @ b21 + b3x
    W65 = g('c6_w') @ g('c5_w')
    b65 = g('c6_w') @ g('c5_b') + g('c6_b')
    W64 = W65 @ g('c4_w')
    b64 = W65 @ g('c4_b') + b65
    WF = W64 @ (np.eye(64) + W321)
    bF = W64 @ b321 + b64
    W32 = g('c3_w') @ g('c2_w')
    b32 = g('c3_w') @ g('c2_b') + g('c3_b')
    return (WF.astype(np.float32), bF.astype(np.float32),
            W32.astype(np.float32), b32.astype(np.float32))


def prep_in_maps(inputs):
    """Full inputs -> per-core in_maps for run_bass_kernel_spmd."""
    f32 = np.float32
    WF, bF, W32, b32 = fold_weights(inputs)
    c1_w = np.ascontiguousarray(np.asarray(inputs['c1_w'], f32))
    ci = c1_w[:, 2053:3013]
    # kernel channel order: img1, img2, img3, img0, point, grid, fc_b-row
    c1wT = np.concatenate([
        ci[:, 64:192].T, ci[:, 192:448].T, ci[:, 448:960].T, ci[:, 0:64].T,
        c1_w[:, 2:5].T, c1_w[:, 0:2].T, ci.sum(axis=1)[None, :],
    ], axis=0).astype(f32)                                   # [966, 1024]
    cgT = np.ascontiguousarray(c1_w[:, 5:2053].T)            # [2048, 1024]
    fcwT = np.ascontiguousarray(np.asarray(inputs['fc_w'], f32).T)
    w32T = np.ascontiguousarray(W32.T)
    wfT = np.ascontiguousarray(WF.T)
    grid2 = np.zeros((2, N), f32)
    grid2[0, 0::2] = -0.2
    grid2[0, 1::2] = 0.2
    grid2[1, :] = -0.2

    codes = np.concatenate([np.asarray(inputs['code'], f32),
                            np.asarray(inputs['global_code'], f32)],
                           axis=1)                           # [B, 2048]
    level0 = np.asarray(inputs['level0'], f32)

    imgsT = []
    for i, S in enumerate(IMG_SIZES):
        img = np.asarray(inputs[f'img{i}'], f32)
        C = img.shape[1]
        t = img.reshape(B, C, S * S).transpose(0, 2, 1)      # [B, S*S, C]
        t = np.concatenate([t, np.zeros((B, 1, C), f32)], axis=1)
        imgsT.append(np.ascontiguousarray(t))

    shared = dict(fcwT=fcwT, c1wT=c1wT, cgT=cgT,
                  c1b=np.ascontiguousarray(np.asarray(inputs['c1_b'], f32)),
                  w32T=w32T, b32v=b32, wfT=wfT, bfv=bF,
                  grid2=grid2,
                  fcb=np.ascontiguousarray(np.asarray(inputs['fc_b'], f32)))
    in_maps = []
    for c in range(NCORES):
        sl = slice(c * BPC, (c + 1) * BPC)
        m = dict(shared)
        m['lvl'] = np.ascontiguousarray(level0[sl])
        m['codesT'] = np.ascontiguousarray(codes[sl].T)
        for i in range(4):
            m[f'it{i}'] = imgsT[i][sl]
        in_maps.append(m)
    return in_maps


def assemble(results):
    out = np.zeros((B, 2 * N, 3), np.float32)
    for c in range(NCORES):
        featc = results[c]['feat']                   # [BPC, 6, N]
        for j in range(3):
            out[c * BPC:(c + 1) * BPC, :N, j] = featc[:, 2 * j, :]
            out[c * BPC:(c + 1) * BPC, N:, j] = featc[:, 2 * j + 1, :]
    return out


_NC_CACHE = None


def get_nc():
    global _NC_CACHE
    if _NC_CACHE is None:
        _NC_CACHE = build_nc()
    return _NC_CACHE


def kernel(**inputs):
    global LAST_RESULTS
    nc = get_nc()
    in_maps = prep_in_maps(inputs)
    res = run_bass_kernel_spmd(nc, in_maps, core_ids=list(range(NCORES)),
                               trace=TRACE)
    LAST_RESULTS = res
    return assemble(res.results)


# revision 10
# speedup vs baseline: 2.7086x; 2.7086x over previous
"""Trainium2 Bass kernel for nn_PartRefinement.

Strategy (pure data parallel, 4 samples per core x 8 cores):

The reference's bilinear sampling is degenerate: with integer-cast weights,
only w11 = (x2-x1)*(y2-y1) in {0,1} survives, so projection is a single-pixel
gather masked by {0,1}.  We store images transposed ([S*S+1, C], zero row at
index S*S) in HBM and use dma_gather with index = masked ? x1*S+y1 : S*S.
The gather output [128, 8, C] is exactly the lhsT layout (points on
partitions) needed by the fc matmul (fc contracts over the point axis).

Everything after the single relu folds into two matmuls on host:
  f1  = c1_w_perm @ [img_fc ; point ; grid ; fc_b] + (c1_cg @ codes + c1_b)
  f1r = relu(W32 @ f1 + b32)          W32 = c3_w @ c2_w
  feat= WF @ f1r + bF                 WF folds w2d1/w2d2/w2d3/c4/c5/c6 chain
"""
import sys
from contextlib import ExitStack

import numpy as np
import ml_dtypes

BF = ml_dtypes.bfloat16

try:
    from concourse import bass, bacc, mybir, tile
except ImportError:  # fresh env without the axon site paths
    sys.path.insert(0, "/opt/trn_rl_repo")
    from concourse import bass, bacc, mybir, tile

from concourse.bass_utils import run_bass_kernel_spmd

F32 = mybir.dt.float32
BF16 = mybir.dt.bfloat16
I32 = mybir.dt.int32
I16 = mybir.dt.int16

B, N = 32, 1024
NCORES = 8
BPC = B // NCORES                     # samples per core
IMG_SIZES = [56, 28, 14, 7]
IMG_CH = [64, 128, 256, 512]
GCH = [128, 128, 256, 512]  # gather elem channels (img0 padded for 256B rule)
NH = 512                              # moving free dim (half of N)

# fc m-chunk -> (gather scale, column offset, width) in kernel channel order
# [img1(128) | img2(256) | img3(512) | img0(64)+point(3)+grid(2)+fcb(1)]
MCHUNKS = [
    (1, 0, 128), (2, 0, 128), (2, 128, 128),
    (3, 0, 128), (3, 128, 128), (3, 256, 128), (3, 384, 128),
    (0, 0, 64),
]

LAST_RESULTS = None                   # BassKernelResults of the last run
TRACE = False


def build_nc():
    nc = bacc.Bacc("TRN2", target_bir_lowering=False, debug=False)

    lvl = nc.declare_dram_parameter("lvl", [BPC, 3, N], F32, isOutput=False)
    lvl_bf = nc.declare_dram_parameter("lvl_bf", [BPC, 3, N], BF16, isOutput=False)
    its = [
        nc.declare_dram_parameter(
            f"it{i}", [BPC, IMG_SIZES[i] * IMG_SIZES[i] + 1, GCH[i]], BF16,
            isOutput=False)
        for i in range(4)
    ]
    fcwT = nc.declare_dram_parameter("fcwT", [1024, 1024], BF16, isOutput=False)
    c1wT = nc.declare_dram_parameter("c1wT", [966, 1024], BF16, isOutput=False)
    cgT = nc.declare_dram_parameter("cgT", [2048, 1024], F32, isOutput=False)
    codesT = nc.declare_dram_parameter("codesT", [2048, BPC], F32, isOutput=False)
    c1b = nc.declare_dram_parameter("c1b", [1024], F32, isOutput=False)
    w32T = nc.declare_dram_parameter("w32T", [1024, 64], BF16, isOutput=False)
    b32v = nc.declare_dram_parameter("b32v", [64], F32, isOutput=False)
    wfT = nc.declare_dram_parameter("wfT", [64, 6], BF16, isOutput=False)
    bfv = nc.declare_dram_parameter("bfv", [6], F32, isOutput=False)
    grid2 = nc.declare_dram_parameter("grid2", [2, N], BF16, isOutput=False)
    fcb = nc.declare_dram_parameter("fcb", [N], BF16, isOutput=False)
    feat = nc.declare_dram_parameter("feat", [BPC, 6, N], F32, isOutput=True)

    add, mult, subtract = (mybir.AluOpType.add, mybir.AluOpType.mult,
                           mybir.AluOpType.subtract)
    is_gt = mybir.AluOpType.is_gt
    amax, amin = mybir.AluOpType.max, mybir.AluOpType.min

    with tile.TileContext(nc) as tc, ExitStack() as es:
        def pool(name, bufs, space="SBUF"):
            return es.enter_context(
                tc.tile_pool(name=name, bufs=bufs, space=space))

        wp = pool("weights", 1)
        cgp = pool("cgpool", 3)
        scr = pool("scratch", 1)
        gp = pool("gather", 2)
        yep = pool("yext", 1)
        f1sb = pool("f1sb", 1)
        outp = pool("outsb", 2)
        psA = pool("psA", 2, "PSUM")
        psB = pool("psB", 2, "PSUM")
        psC = pool("psC", 1, "PSUM")
        psD = pool("psD", 1, "PSUM")
        psE = pool("psE", 2, "PSUM")

        # ---- persistent weights -------------------------------------------
        fcw_t = wp.tile([128, 8, 1024], BF16, name="fcw")
        nc.sync.dma_start(out=fcw_t[:], in_=fcwT[:].rearrange("(k p) o -> p k o", p=128))
        c1wa = wp.tile([128, 7, 1024], BF16, name="c1wa")
        nc.sync.dma_start(out=c1wa[:],
                          in_=c1wT[0:896].rearrange("(k p) o -> p k o", p=128))
        c1wb = wp.tile([70, 1024], BF16, name="c1wb")
        nc.sync.dma_start(out=c1wb[:], in_=c1wT[896:966])
        w32_t = wp.tile([128, 8, 64], BF16, name="w32t")
        nc.sync.dma_start(out=w32_t[:], in_=w32T[:].rearrange("(k p) m -> p k m", p=128))
        wf_t = wp.tile([64, 6], BF16, name="wft")
        nc.sync.dma_start(out=wf_t[:], in_=wfT[:])
        b32_t = wp.tile([64, 1], F32, name="b32t")
        nc.sync.dma_start(out=b32_t[:], in_=b32v[:].unsqueeze(1))
        bf_t = wp.tile([6, 1], F32, name="bft")
        nc.sync.dma_start(out=bf_t[:], in_=bfv[:].unsqueeze(1))
        c1b_t = wp.tile([128, 8], F32, name="c1bt")
        nc.sync.dma_start(out=c1b_t[:], in_=c1b[:].rearrange("(m p) -> p m", p=128))
        codes_t = wp.tile([128, 16, BPC], F32, name="codest")
        nc.sync.dma_start(out=codes_t[:],
                          in_=codesT[:].rearrange("(k p) b -> p k b", p=128))

        # ---- coordinates & gather indices ---------------------------------
        # c16[p, c, b, i] = level0[b, c, i*16 + p]  (dma_gather wrap order),
        # then replicated to all 8 groups of 16 partitions.
        c16 = scr.tile([16, 3, BPC, 64], F32, name="c16")
        for c in range(3):
            for b in range(BPC):
                nc.sync.dma_start(out=c16[:, c, b, :],
                                  in_=lvl[b, c].rearrange("(i p) -> p i", p=16))
        coords = scr.tile([128, 3, BPC, 64], F32, name="coords")
        for g in range(8):
            nc.sync.dma_start(out=coords[16 * g:16 * (g + 1)], in_=c16[:])

        NW = BPC * 64                                # free width of coord math
        x0 = coords[:, 0].rearrange("p b i -> p (b i)")
        yy = coords[:, 1].rearrange("p b i -> p (b i)")
        zz = coords[:, 2].rearrange("p b i -> p (b i)")

        def s_t(tag):
            return scr.tile([128, NW], F32, name=tag, tag=tag)

        rz = s_t("rz")
        nc.vector.reciprocal(rz[:], zz)
        hh = s_t("hh")
        ww = s_t("ww")
        tq = s_t("tq")
        tr = s_t("tr")
        for qt, num, dst, sgn in ((tq, yy, hh, 248.0), (tr, x0, ww, -248.0)):
            nc.vector.tensor_tensor(qt[:], num, rz[:], op=mult)     # q0 = n*rz
            # one Newton step of the division: q = q0 + rz*(n - q0*z)
            t1 = s_t("nt1")
            nc.vector.tensor_tensor(t1[:], qt[:], zz, op=mult)
            nc.vector.tensor_tensor(t1[:], num, t1[:], op=subtract)
            nc.vector.tensor_tensor(t1[:], t1[:], rz[:], op=mult)
            nc.vector.tensor_tensor(qt[:], qt[:], t1[:], op=add)
            nc.vector.tensor_scalar(out=dst[:], in0=qt[:], scalar1=sgn,
                                    scalar2=111.5, op0=mult, op1=add)
            nc.vector.tensor_scalar(out=dst[:], in0=dst[:], scalar1=0.0,
                                    scalar2=223.0, op0=amax, op1=amin)

        idx_tiles = []
        for s, S in enumerate(IMG_SIZES):
            fls = []
            ms = []
            for axis, src in ((0, hh), (1, ww)):
                xf = s_t("xf")
                nc.vector.tensor_scalar_mul(xf[:], src[:], float(S / 224.0))
                xi = scr.tile([128, NW], I32, name="xi", tag="xi")
                nc.vector.tensor_copy(xi[:], xf[:])                  # cast
                xq = s_t("xq")
                nc.vector.tensor_copy(xq[:], xi[:])                  # back to f32
                gt = s_t("gt")
                nc.vector.tensor_tensor(gt[:], xq[:], xf[:], op=is_gt)
                fl = s_t(f"fl{axis}")
                nc.vector.tensor_tensor(fl[:], xq[:], gt[:], op=subtract)
                hf = s_t("hf")
                nc.vector.tensor_tensor(hf[:], xf[:], fl[:], op=is_gt)
                x2 = s_t("x2")
                nc.vector.tensor_tensor(x2[:], fl[:], hf[:], op=add)
                nc.vector.tensor_scalar_min(x2[:], x2[:], float(S - 1))
                mx = s_t(f"mx{axis}")
                nc.vector.tensor_tensor(mx[:], x2[:], fl[:], op=subtract)
                fls.append(fl)
                ms.append(mx)
            idxf = s_t("idxf")
            nc.vector.tensor_scalar_mul(idxf[:], fls[0][:], float(S))
            nc.vector.tensor_tensor(idxf[:], idxf[:], fls[1][:], op=add)
            mm = s_t("mm")
            nc.vector.tensor_tensor(mm[:], ms[0][:], ms[1][:], op=mult)
            # idx_eff = m*(idx - S^2) + S^2
            nc.vector.tensor_scalar_sub(idxf[:], idxf[:], float(S * S))
            nc.vector.tensor_tensor(idxf[:], idxf[:], mm[:], op=mult)
            nc.vector.tensor_scalar_add(idxf[:], idxf[:], float(S * S))
            idx16 = scr.tile([128, BPC, 64], I16, name=f"idx{s}", tag=f"idx{s}")
            nc.vector.tensor_copy(idx16.rearrange("p b i -> p (b i)"), idxf[:])
            idx_tiles.append(idx16)

        # ---- gathers -------------------------------------------------------
        g_tiles = [None] * BPC

        def emit_gathers(b):
            tiles = []
            for s in range(4):
                C = GCH[s]
                gt_ = gp.tile([128, 8, C], BF16, name=f"g{s}_{b}", tag=f"g{s}")
                nc.gpsimd.dma_gather(
                    out_ap=gt_[:],
                    in_ap=its[s][b],
                    idxs_ap=idx_tiles[s][:, b, :],
                    num_idxs=N,
                    num_idxs_reg=N,
                    elem_size=C,
                    queue_num=0,
                )
                tiles.append(gt_)
            g_tiles[b] = tiles

        emit_gathers(0)
        emit_gathers(1)

        # ---- CT = c1_cg @ [code; global_code] + c1_b  (all samples) -------
        # PSUM holds only one k-step (per-matmul start/stop); accumulate in
        # SBUF to avoid 8 interleaved accumulation groups in one bank.
        ct_sb = wp.tile([128, 8, BPC], F32, name="ctsb")
        nc.vector.memset(ct_sb[:], 0.0)
        for k in range(16):
            cg_t = cgp.tile([128, 1024], F32, name=f"cg{k}", tag="cg")
            nc.sync.dma_start(out=cg_t[:], in_=cgT[128 * k:128 * (k + 1)])
            ctp = psE.tile([128, 8, BPC], F32, name="ctp", tag="ctp")
            for m in range(8):
                nc.tensor.matmul(
                    ctp[:, m, :], (cg_t[:, 128 * m:128 * (m + 1)]),
                    (codes_t[:, k, :]), start=True, stop=True,
                )
            acc_v = ct_sb.rearrange("p m b -> p (m b)")
            nc.vector.tensor_tensor(acc_v, acc_v,
                                    ctp.rearrange("p m b -> p (m b)"), op=add)
        for m in range(8):
            nc.vector.tensor_scalar(out=ct_sb[:, m, :], in0=ct_sb[:, m, :],
                                    scalar1=c1b_t[:, m:m + 1], scalar2=None,
                                    op0=add)

        # ---- per-sample pipeline ------------------------------------------
        for b in range(BPC):
            g0, g1, g2, g3 = g_tiles[b]
            gmap = {0: g0, 1: g1, 2: g2, 3: g3}
            for oc in range(2):
                osl = slice(oc * NH, (oc + 1) * NH)
                # fc: Y[c, o] = sum_n GT[n, c] fcwT[n, o]
                ye = [yep.tile([128, NH], BF16, name=f"ye{k}", tag=f"ye{k}")
                      for k in range(7)]
                ye7 = yep.tile([70, NH], BF16, name="ye7", tag="ye7")
                for m, (s, c0, cw) in enumerate(MCHUNKS):
                    yp = psA.tile([cw, NH], F32, name="yp", tag="yp")
                    for k in range(8):
                        nc.tensor.matmul(
                            yp[:], (gmap[s][:, k, c0:c0 + cw]),
                            (fcw_t[:, k, osl]),
                            start=(k == 0), stop=(k == 7),
                        )
                    if m < 7:
                        nc.vector.tensor_copy(ye[m][:], yp[:])
                    else:
                        nc.vector.tensor_copy(ye7[0:64, :], yp[:])
                nc.sync.dma_start(out=ye7[64:67, :], in_=lvl_bf[b][:, osl])
                nc.sync.dma_start(out=ye7[67:69, :], in_=grid2[:, osl])
                nc.sync.dma_start(out=ye7[69:70, :], in_=fcb[osl].unsqueeze(0))

                # c1
                f1 = [f1sb.tile([128, NH], BF16, name=f"f1_{m}", tag=f"f1_{m}")
                      for m in range(8)]
                for m in range(8):
                    fp1 = psB.tile([128, NH], F32, name="f1p", tag="f1p")
                    for k in range(8):
                        if k < 7:
                            nc.tensor.matmul(
                                fp1[:], (c1wa[:, k, 128 * m:128 * (m + 1)]),
                                (ye[k][:]), start=(k == 0), stop=False)
                        else:
                            nc.tensor.matmul(
                                fp1[:], (c1wb[:, 128 * m:128 * (m + 1)]),
                                (ye7[:]), start=False, stop=True)
                    nc.vector.tensor_scalar(out=f1[m][:], in0=fp1[:],
                                            scalar1=ct_sb[:, m, b:b + 1],
                                            scalar2=None, op0=add)

                # W32 + relu
                rp = psC.tile([64, NH], F32, name="rp", tag="rp")
                for k in range(8):
                    nc.tensor.matmul(rp[:], (w32_t[:, k, :]), (f1[k][:]),
                                     start=(k == 0), stop=(k == 7))
                f1r = outp.tile([64, NH], BF16, name="f1r", tag="f1r")
                nc.scalar.activation(f1r[:], rp[:],
                                     mybir.ActivationFunctionType.Relu,
                                     bias=b32_t[:], scale=1.0)

                # WF
                fpp = psD.tile([6, NH], F32, name="fpp", tag="fpp")
                nc.tensor.matmul(fpp[:], (wf_t[:]), (f1r[:]))
                feat_sb = outp.tile([6, NH], F32, name="featsb", tag="featsb")
                nc.vector.tensor_scalar(out=feat_sb[:], in0=fpp[:],
                                        scalar1=bf_t[:], scalar2=None, op0=add)
                nc.sync.dma_start(out=feat[b][:, osl], in_=feat_sb[:])

            if b + 2 < BPC:
                emit_gathers(b + 2)

    nc.compile()
    return nc


def fold_weights(inp):
    f64 = np.float64
    g = lambda k: np.asarray(inp[k], f64)
    w2d1s = g('w2d1').sum(-1)
    W21 = g('w2d2') @ w2d1s
    b21 = g('w2d2') @ g('b2d1') + g('b2d2')
    BD3 = np.zeros((64, 128), f64)
    for u in range(2):
        BD3[np.arange(32) * 2 + u, u * 64:(u + 1) * 64] = g('w2d3')
    b3x = np.zeros(64, f64)
    b3x[0::2] = g('b2d3'); b3x[1::2] = g('b2d3')
    W321 = BD3 @ W21
    b321 = BD3 @ b21 + b3x
    W65 = g('c6_w') @ g('c5_w')
    b65 = g('c6_w') @ g('c5_b') + g('c6_b')
    W64 = W65 @ g('c4_w')
    b64 = W65 @ g('c4_b') + b65
    WF = W64 @ (np.eye(64) + W321)
    bF = W64 @ b321 + b64
    W32 = g('c3_w') @ g('c2_w')
    b32 = g('c3_w') @ g('c2_b') + g('c3_b')
    return (WF.astype(np.float32), bF.astype(np.float32),
            W32.astype(np.float32), b32.astype(np.float32))


def prep_in_maps(inputs):
    """Full inputs -> per-core in_maps for run_bass_kernel_spmd."""
    f32 = np.float32
    WF, bF, W32, b32 = fold_weights(inputs)
    c1_w = np.ascontiguousarray(np.asarray(inputs['c1_w'], f32))
    ci = c1_w[:, 2053:3013]
    # kernel channel order: img1, img2, img3, img0, point, grid, fc_b-row
    c1wT = np.concatenate([
        ci[:, 64:192].T, ci[:, 192:448].T, ci[:, 448:960].T, ci[:, 0:64].T,
        c1_w[:, 2:5].T, c1_w[:, 0:2].T, ci.sum(axis=1)[None, :],
    ], axis=0).astype(BF)                                    # [966, 1024]
    cgT = np.ascontiguousarray(c1_w[:, 5:2053].T)            # [2048, 1024]
    fcwT = np.ascontiguousarray(np.asarray(inputs['fc_w'], f32).T.astype(BF))
    w32T = np.ascontiguousarray(W32.T.astype(BF))
    wfT = np.ascontiguousarray(WF.T.astype(BF))
    grid2 = np.zeros((2, N), BF)
    grid2[0, 0::2] = BF(-0.2)
    grid2[0, 1::2] = BF(0.2)
    grid2[1, :] = BF(-0.2)

    codes = np.concatenate([np.asarray(inputs['code'], f32),
                            np.asarray(inputs['global_code'], f32)],
                           axis=1)                           # [B, 2048]
    level0 = np.asarray(inputs['level0'], f32)

    imgsT = []
    for i, S in enumerate(IMG_SIZES):
        img = np.asarray(inputs[f'img{i}'], f32)
        C = img.shape[1]
        t = img.reshape(B, C, S * S).transpose(0, 2, 1).astype(BF)
        full = np.zeros((B, S * S + 1, GCH[i]), BF)          # zero row + pad
        full[:, :S * S, :C] = t
        imgsT.append(full)

    shared = dict(fcwT=fcwT, c1wT=c1wT, cgT=cgT,
                  c1b=np.ascontiguousarray(np.asarray(inputs['c1_b'], f32)),
                  w32T=w32T, b32v=b32, wfT=wfT, bfv=bF,
                  grid2=grid2,
                  fcb=np.ascontiguousarray(
                      np.asarray(inputs['fc_b'], f32).astype(BF)))
    in_maps = []
    for c in range(NCORES):
        sl = slice(c * BPC, (c + 1) * BPC)
        m = dict(shared)
        m['lvl'] = np.ascontiguousarray(level0[sl])
        m['lvl_bf'] = np.ascontiguousarray(level0[sl].astype(BF))
        m['codesT'] = np.ascontiguousarray(codes[sl].T)
        for i in range(4):
            m[f'it{i}'] = imgsT[i][sl]
        in_maps.append(m)
    return in_maps


def assemble(results):
    out = np.zeros((B, 2 * N, 3), np.float32)
    for c in range(NCORES):
        featc = results[c]['feat']                   # [BPC, 6, N]
        for j in range(3):
            out[c * BPC:(c + 1) * BPC, :N, j] = featc[:, 2 * j, :]
            out[c * BPC:(c + 1) * BPC, N:, j] = featc[:, 2 * j + 1, :]
    return out


_NC_CACHE = None


def get_nc():
    global _NC_CACHE
    if _NC_CACHE is None:
        _NC_CACHE = build_nc()
    return _NC_CACHE


def kernel(**inputs):
    global LAST_RESULTS
    nc = get_nc()
    in_maps = prep_in_maps(inputs)
    res = run_bass_kernel_spmd(nc, in_maps, core_ids=list(range(NCORES)),
                               trace=TRACE)
    LAST_RESULTS = res
    return assemble(res.results)
